# revision 4
# baseline (speedup 1.0000x reference)
"""Trainium2 Bass kernel: two-phase LSTM encoder.

Computes, for batch B=4096, hidden H=1024:
  scan1: 8 steps of (Linear(2->256) + LSTMCell) over obs_traj_rel, carry (h0, c0)
  c_out = h1.T.reshape(B, H)
  scan2: 12 steps over pre_traj_rel, carry (h1, c1)
  x_out = h2.T.reshape(B, H)

Strategy (data-parallel over batch, 8 NeuronCores, BL=512 rows each):
  - The 256-wide input embedding is folded into the gate weights on the host:
      gates = x @ (W_ih @ W_in).T + h @ W_hh.T + (W_ih @ b_in + b)
    so the per-step device work is one K=2 matmul + one K=1024 matmul.
  - Hidden state lives in SBUF transposed ([H, BL]) so it is directly the
    moving operand of the gate matmul; gates come out as [4H, BL] tiles.
  - Matmul inputs in bf16 (full PE rate), PSUM accumulation fp32, gate
    activations on ScalarE (sigmoid/tanh share one table set) reading PSUM
    with the gate bias folded in as the per-partition activation bias,
    cell-state update on VectorE (c in fp32, activations/products in bf16).
  - The four K=2 input-projection matmuls of a j-group are row-packed into
    disjoint 32-row strips of the PE array (tile_position=(32r,0)) and are
    HOISTED into the middle of the previous j-group's o-gate matmul stream,
    so their LDWEIGHTS traffic spreads out instead of bursting at the group
    boundary (which measured as a ~104ns stretch of every group's first MM).
  - Activations and the cell-state update run in two column halves so the
    end-of-step h_next latency (which gates the next step's last k-block
    matmul) is roughly halved; intermediates are bf16 for DVE/ACT speed.
  - Weight columns are permuted on the host so each j-group's four gate
    blocks (f,i,g,o for rows 128j..128j+127 of H) are one contiguous
    512-column chunk; weights DMA in (j: wx-chunk + 8 k-chunks) order so
    compute starts after ~1.2MB instead of the full 8.4MB.
  - h is written back as bf16 (next step's matmul operand); the scan-final h
    is additionally produced in fp32 and DMA'd out as [H, BL]; the host
    concatenation of those per-core blocks is exactly h.T, so c_out/x_out are
    free reshapes.
"""

import numpy as np
import ml_dtypes

T_OBS, T_PRE, B = 8, 12, 4096
FEAT, H = 2, 1024
N_CORES = 8
BL = B // N_CORES        # 512 batch rows per core
KB = H // 128            # 8 contraction blocks over H
NG = 4 * H // 128        # 32 gate row-tiles over 4H
HALF = BL // 2

# Gate-block permutation: new column block p = 4*j + gi holds original gate
# block (f,i,g,o)[gi] for H-rows j*128..(j+1)*128, i.e. original m-index
# _PERM[p]. Gates appear in (f,i,g,o) order so the forget gate (first in the
# cell-update chain) finishes earliest.
_GATE_ORIG = (1, 0, 2, 3)  # f,i,g,o -> original gate index (i,f,g,o order)
_PERM = [g0 * KB + j for j in range(KB) for g0 in _GATE_ORIG]

_BF16 = ml_dtypes.bfloat16
_CACHE = {}


def _build_nc():
    import concourse.tile as tile
    from concourse import bacc, mybir

    f32 = mybir.dt.float32
    bf16 = mybir.dt.bfloat16
    SIG = mybir.ActivationFunctionType.Sigmoid
    TANH = mybir.ActivationFunctionType.Tanh

    nc = bacc.Bacc(
        "TRN2", target_bir_lowering=False, debug=False, enable_asserts=False
    )

    scans = ("obs", "pre")
    d_w = [
        nc.dram_tensor(f"w_{s}", [H, 4 * H], bf16, kind="ExternalInput").ap()
        for s in scans
    ]
    d_wx = [
        nc.dram_tensor(f"wx_{s}", [128, 4 * H], bf16, kind="ExternalInput").ap()
        for s in scans
    ]
    d_bias = [
        nc.dram_tensor(f"bias_{s}", [128, NG], f32, kind="ExternalInput").ap()
        for s in scans
    ]
    d_x = [
        nc.dram_tensor(f"x_{s}", [t, FEAT, BL], bf16, kind="ExternalInput").ap()
        for s, t in zip(scans, (T_OBS, T_PRE))
    ]
    d_h0 = nc.dram_tensor("h0T", [H, BL], bf16, kind="ExternalInput").ap()
    d_c0 = nc.dram_tensor("c0T", [H, BL], f32, kind="ExternalInput").ap()
    d_c1 = nc.dram_tensor("c1T", [H, BL], f32, kind="ExternalInput").ap()
    d_hout = [
        nc.dram_tensor(f"h{i}T", [H, BL], f32, kind="ExternalOutput").ap()
        for i in (1, 2)
    ]

    with tile.TileContext(nc) as tc:
        with (
            tc.tile_pool(name="wp", bufs=1) as wp,
            tc.tile_pool(name="hp", bufs=18) as hp,
            tc.tile_pool(name="cp", bufs=1) as cp,
            tc.tile_pool(name="gf", bufs=4) as gf,
            tc.tile_pool(name="gb", bufs=14) as gb,
            tc.tile_pool(name="xp", bufs=4) as xp,
            tc.tile_pool(name="pp", bufs=8, space="PSUM") as pp,
        ):
            # Persistent weights. Scan-1 set loads in (j-chunk)-major order so
            # the first j-group can start after ~1.2MB; scan-2 set is emitted
            # inside the scan loop (after step 1) so its DMA traffic queues
            # behind the critical first-step loads.
            w_sb = [
                [
                    wp.tile([128, 4 * H], bf16, tag=f"w{s}_{k}", name=f"w{s}_{k}")
                    for k in range(KB)
                ]
                for s in range(2)
            ]
            wx_sb = [
                wp.tile([128, 4 * H], bf16, tag=f"wx{s}", name=f"wx{s}")
                for s in range(2)
            ]
            bias_sb = [
                wp.tile([128, NG], f32, tag=f"bias{s}", name=f"bias{s}")
                for s in range(2)
            ]

            def load_weight_set(s):
                # bias first (tiny), then per j-chunk: the wx strip columns
                # and the 8 k-chunks, so j-group 0's full working set lands
                # first. All on the SP queue; h/c/x loads use other engines'
                # queues so they don't wait behind these.
                nc.sync.dma_start(out=bias_sb[s], in_=d_bias[s][:, :])
                for j in range(KB):
                    jsl = slice(j * 512, (j + 1) * 512)
                    nc.sync.dma_start(out=wx_sb[s][:, jsl], in_=d_wx[s][:, jsl])
                    for k in range(KB):
                        nc.sync.dma_start(
                            out=w_sb[s][k][:, jsl],
                            in_=d_w[s][k * 128 : (k + 1) * 128, jsl],
                        )

            # PE warm-up: the array sits idle ~8us waiting for the first
            # weight/h0 DMAs, and would then start the real stream at the
            # HAM-throttled 1.2 GHz clock. Dummy matmuls on zeroed tiles
            # (never read) during that window flip the clock gate to 8/8
            # before the real stream begins (~3.4us of PE-busy needed).
            wu_w = xp.tile([128, 128], bf16, tag="x", name="wu_w")
            wu_rhs = xp.tile([128, BL], bf16, tag="x", name="wu_rhs")
            nc.vector.memset(wu_w, 0.0)
            nc.vector.memset(wu_rhs, 0.0)
            wu_p = pp.tile([128, BL], f32, tag="ps", name="wu_p")
            for _ in range(12):
                nc.tensor.matmul(wu_p, wu_w, wu_rhs, start=True, stop=True)

            load_weight_set(0)

            # h0 first (needed by the very first k-loop within ~2us of
            # compute start), then c0 (only needed at activation time).
            h_cur = [
                hp.tile([128, BL], bf16, tag="h", name=f"h_init_{k}")
                for k in range(KB)
            ]
            for k in range(KB):
                nc.scalar.dma_start(out=h_cur[k], in_=d_h0[k * 128 : (k + 1) * 128, :])
            c_sb = [
                cp.tile([128, BL], f32, tag=f"c{j}", name=f"c{j}") for j in range(KB)
            ]
            for j in range(KB):
                nc.scalar.dma_start(out=c_sb[j], in_=d_c0[j * 128 : (j + 1) * 128, :])

            steps = [(0, t) for t in range(T_OBS)] + [(1, t) for t in range(T_PRE)]
            x_tiles = {}

            def load_x(s, t):
                if (s, t) in x_tiles:
                    return x_tiles[(s, t)]
                xt = xp.tile([128, BL], bf16, tag="x", name=f"x_{s}_{t}")
                for r in range(4):
                    nc.gpsimd.dma_start(
                        out=xt[32 * r : 32 * r + FEAT, :], in_=d_x[s][t, :, :]
                    )
                x_tiles[(s, t)] = xt
                return xt

            def emit_input_mms(s, t, g, xt):
                # four K=2 input-projection matmuls, row-packed into disjoint
                # 32-row strips -> concurrent on the PE. Starts group g's
                # PSUM accumulation.
                ps = []
                for gi in range(4):
                    m = 4 * g + gi
                    p = pp.tile([128, BL], f32, tag="ps", name=f"ps_{s}_{t}_{m}")
                    msl = slice(m * 128, (m + 1) * 128)
                    rsl = slice(32 * gi, 32 * gi + FEAT)
                    nc.tensor.matmul(
                        p,
                        wx_sb[s][rsl, msl],
                        xt[rsl, :],
                        start=True,
                        stop=False,
                        tile_position=(32 * gi, 0),
                        skip_group_check=True,
                    )
                    ps.append(p)
                return ps

            HA, HB = slice(0, HALF), slice(HALF, BL)
            pending = None  # psum tiles of the next group, emitted early

            for si_, (s, t) in enumerate(steps):
                if s == 0 and t == 1:
                    load_weight_set(1)
                xt = load_x(s, t)
                if si_ + 1 < len(steps):
                    load_x(*steps[si_ + 1])  # prefetch next step's x

                T = T_OBS if s == 0 else T_PRE
                last = t == T - 1
                produce_next = not (s == 1 and last)
                d_out = d_hout[s]
                h_next = [None] * KB

                for g in range(KB):
                    if pending is None:  # very first group only
                        pending = emit_input_mms(s, t, g, xt)
                    ps = pending
                    pending = None

                    ms = [4 * g + gi for gi in range(4)]
                    for gi in range(4):
                        msl = slice(ms[gi] * 128, (ms[gi] + 1) * 128)
                        for k in range(KB):
                            nc.tensor.matmul(
                                ps[gi],
                                w_sb[s][k][:, msl],
                                h_cur[k],
                                start=False,
                                stop=(k == KB - 1),
                                skip_group_check=True,
                            )
                            # hoist the next group's input MMs into the
                            # o-gate stream so their LDWEIGHTS don't burst
                            # at the group boundary
                            if gi == 3 and k == 0:
                                if g < KB - 1:
                                    pending = emit_input_mms(s, t, g + 1, xt)
                                elif si_ + 1 < len(steps):
                                    s2, t2 = steps[si_ + 1]
                                    pending = emit_input_mms(
                                        s2, t2, 0, x_tiles[(s2, t2)]
                                    )
                    pf, pi, pg, po = ps
                    bs = bias_sb[s]

                    def _act(p_in, m, func, nm, hs):
                        o = _act.tiles.get(nm)
                        if o is None:
                            o = gb.tile([128, BL], bf16, tag="g", name=nm)
                            _act.tiles[nm] = o
                        nc.scalar.activation(
                            out=o[:, hs], in_=p_in[:, hs], func=func,
                            bias=bs[:, m : m + 1],
                        )
                        return o

                    _act.tiles = {}
                    # f,i,g gate activations in column halves (bf16 out)
                    for hs in (HA, HB):
                        sf = _act(pf, ms[0], SIG, f"sf_{s}_{t}_{g}", hs)
                    for hs in (HA, HB):
                        si = _act(pi, ms[1], SIG, f"si_{s}_{t}_{g}", hs)
                    for hs in (HA, HB):
                        tg = _act(pg, ms[2], TANH, f"tg_{s}_{t}_{g}", hs)

                    # cell update per half: c = sf*c + si*tg (c stays fp32)
                    t1 = gf.tile([128, BL], f32, tag="t", name=f"t1_{s}_{t}_{g}")
                    t2 = gb.tile([128, BL], bf16, tag="g", name=f"t2_{s}_{t}_{g}")
                    for hs in (HA, HB):
                        nc.vector.tensor_mul(t1[:, hs], sf[:, hs], c_sb[g][:, hs])
                        nc.vector.tensor_mul(t2[:, hs], si[:, hs], tg[:, hs])
                        nc.vector.tensor_add(c_sb[g][:, hs], t1[:, hs], t2[:, hs])

                    # o-gate + tanh(c) per half; so_a/tc_a complete before
                    # so_b/tc_b so the first half of h_next lands early
                    so = _act.tiles.get(f"so_{s}_{t}_{g}")
                    tc_j = gb.tile([128, BL], bf16, tag="g", name=f"tc_{s}_{t}_{g}")
                    h_n = (
                        hp.tile([128, BL], bf16, tag="h", name=f"h_{s}_{t}_{g}")
                        if produce_next
                        else None
                    )
                    hf = (
                        gf.tile([128, BL], f32, tag="t", name=f"hf_{s}_{g}")
                        if last
                        else None
                    )
                    for hs in (HA, HB):
                        so = _act(po, ms[3], SIG, f"so_{s}_{t}_{g}", hs)
                        nc.scalar.activation(
                            out=tc_j[:, hs], in_=c_sb[g][:, hs], func=TANH
                        )
                        # next-step h first: it is on the critical path; the
                        # fp32 output copy and its DMA are not.
                        if produce_next:
                            nc.vector.tensor_mul(h_n[:, hs], so[:, hs], tc_j[:, hs])
                        if last:
                            nc.vector.tensor_mul(hf[:, hs], so[:, hs], tc_j[:, hs])
                            nc.sync.dma_start(
                                out=d_out[g * 128 : (g + 1) * 128, hs], in_=hf[:, hs]
                            )
                    if produce_next:
                        h_next[g] = h_n

                    # refresh cell state for scan 2 as soon as scan 1 is done
                    # reading this j-block (lands during the remaining steps)
                    if s == 0 and last:
                        nc.scalar.dma_start(
                            out=c_sb[g], in_=d_c1[g * 128 : (g + 1) * 128, :]
                        )

                if produce_next:
                    h_cur = h_next

    nc.compile()
    return nc


def _prep_host(inputs):
    inputs = {k: np.asarray(v) for k, v in inputs.items()}
    f32 = np.float32
    W_in = inputs["W_in"].astype(np.float64)
    b_in = inputs["b_in"].astype(np.float64)

    shared = {}
    for tag in ("obs", "pre"):
        W_ih = inputs[f"W_ih_{tag}"].astype(np.float64)
        W_hh = inputs[f"W_hh_{tag}"].astype(f32)
        b = inputs[f"b_{tag}"].astype(np.float64)
        W_eff = (W_ih @ W_in).astype(f32)        # [4H, FEAT]
        b_eff = (W_ih @ b_in + b).astype(f32)    # [4H]
        # permute gate blocks into j-grouped (f,i,g,o) order
        w_p = W_hh.T.reshape(H, NG, 128)[:, _PERM, :].reshape(H, 4 * H)
        wx_p = np.zeros((128, 4 * H), f32)
        for r in range(4):
            wx_p[32 * r : 32 * r + FEAT] = (
                W_eff.T.reshape(FEAT, NG, 128)[:, _PERM, :].reshape(FEAT, 4 * H)
            )
        bias_p = b_eff.reshape(NG, 128)[_PERM, :].T  # [128, NG]
        shared[f"w_{tag}"] = np.ascontiguousarray(w_p).astype(_BF16)
        shared[f"wx_{tag}"] = np.ascontiguousarray(wx_p).astype(_BF16)
        shared[f"bias_{tag}"] = np.ascontiguousarray(bias_p)

    obs = inputs["obs_traj_rel"].astype(f32)
    pre = inputs["pre_traj_rel"].astype(f32)
    h0 = inputs["h0"].astype(f32)
    c0 = inputs["c0"].astype(f32)
    c1 = inputs["c1"].astype(f32)

    in_maps = []
    for c in range(N_CORES):
        sl = slice(c * BL, (c + 1) * BL)
        m = dict(shared)
        m["x_obs"] = np.ascontiguousarray(obs[:, sl, :].transpose(0, 2, 1)).astype(
            _BF16
        )
        m["x_pre"] = np.ascontiguousarray(pre[:, sl, :].transpose(0, 2, 1)).astype(
            _BF16
        )
        m["h0T"] = np.ascontiguousarray(h0[sl].T).astype(_BF16)
        m["c0T"] = np.ascontiguousarray(c0[sl].T)
        m["c1T"] = np.ascontiguousarray(c1[sl].T)
        in_maps.append(m)
    return in_maps


def _run(inputs, trace=False):
    from concourse import bass_utils

    nc = _CACHE.get("nc")
    if nc is None:
        nc = _build_nc()
        _CACHE["nc"] = nc
    in_maps = _prep_host(inputs)
    res = bass_utils.run_bass_kernel_spmd(
        nc, in_maps, core_ids=list(range(N_CORES)), trace=trace
    )
    h1 = np.concatenate([r["h1T"] for r in res.results], axis=1)  # [H, B] == h1.T
    h2 = np.concatenate([r["h2T"] for r in res.results], axis=1)
    c_out = np.ascontiguousarray(h1.reshape(B, H), dtype=np.float32)
    x_out = np.ascontiguousarray(h2.reshape(B, H), dtype=np.float32)
    return (c_out, x_out), res


def kernel(**inputs):
    out, _ = _run(inputs, trace=False)
    return out


# revision 11
# speedup vs baseline: 1.0020x; 1.0020x over previous
"""Trainium2 Bass kernel: two-phase LSTM encoder.

Computes, for batch B=4096, hidden H=1024:
  scan1: 8 steps of (Linear(2->256) + LSTMCell) over obs_traj_rel, carry (h0, c0)
  c_out = h1.T.reshape(B, H)
  scan2: 12 steps over pre_traj_rel, carry (h1, c1)
  x_out = h2.T.reshape(B, H)

Strategy (data-parallel over batch, 8 NeuronCores, BL=512 rows each):
  - The 256-wide input embedding is folded into the gate weights on the host:
      gates = x @ (W_ih @ W_in).T + h @ W_hh.T + (W_ih @ b_in + b)
    so the per-step device work is one K=2 matmul + one K=1024 matmul.
  - Hidden state lives in SBUF transposed ([H, BL]) so it is directly the
    moving operand of the gate matmul; gates come out as [4H, BL] tiles.
  - Matmul inputs in bf16 (full PE rate), PSUM accumulation fp32, gate
    activations on ScalarE (sigmoid/tanh share one table set) reading PSUM
    with the gate bias folded in as the per-partition activation bias,
    cell-state update on VectorE (c in fp32, activations/products in bf16).
  - The four K=2 input-projection matmuls of a j-group are row-packed into
    disjoint 32-row strips of the PE array (tile_position=(32r,0)) and are
    HOISTED into the middle of the previous j-group's o-gate matmul stream,
    so their LDWEIGHTS traffic spreads out instead of bursting at the group
    boundary (which measured as a ~104ns stretch of every group's first MM).
  - Activations and the cell-state update run in two column halves so the
    end-of-step h_next latency (which gates the next step's last k-block
    matmul) is roughly halved; intermediates are bf16 for DVE/ACT speed.
  - Weight columns are permuted on the host so each j-group's four gate
    blocks (f,i,g,o for rows 128j..128j+127 of H) are one contiguous
    512-column chunk; weights DMA in (j: wx-chunk + 8 k-chunks) order so
    compute starts after ~1.2MB instead of the full 8.4MB.
  - h is written back as bf16 (next step's matmul operand); the scan-final h
    is additionally produced in fp32 and DMA'd out as [H, BL]; the host
    concatenation of those per-core blocks is exactly h.T, so c_out/x_out are
    free reshapes.
"""

import numpy as np
import ml_dtypes

T_OBS, T_PRE, B = 8, 12, 4096
FEAT, H = 2, 1024
N_CORES = 8
BL = B // N_CORES        # 512 batch rows per core
KB = H // 128            # 8 contraction blocks over H
NG = 4 * H // 128        # 32 gate row-tiles over 4H
HALF = BL // 2

# Gate-block permutation: new column block p = 4*j + gi holds original gate
# block (f,i,g,o)[gi] for H-rows j*128..(j+1)*128, i.e. original m-index
# _PERM[p]. Gates appear in (f,i,g,o) order so the forget gate (first in the
# cell-update chain) finishes earliest.
_GATE_ORIG = (1, 0, 2, 3)  # f,i,g,o -> original gate index (i,f,g,o order)
_PERM = [g0 * KB + j for j in range(KB) for g0 in _GATE_ORIG]

_BF16 = ml_dtypes.bfloat16
_CACHE = {}


def _build_nc():
    import concourse.tile as tile
    from concourse import bacc, mybir

    f32 = mybir.dt.float32
    bf16 = mybir.dt.bfloat16
    SIG = mybir.ActivationFunctionType.Sigmoid
    TANH = mybir.ActivationFunctionType.Tanh

    nc = bacc.Bacc(
        "TRN2", target_bir_lowering=False, debug=False, enable_asserts=False
    )

    scans = ("obs", "pre")
    d_w = [
        nc.dram_tensor(f"w_{s}", [H, 4 * H], bf16, kind="ExternalInput").ap()
        for s in scans
    ]
    d_wx = [
        nc.dram_tensor(f"wx_{s}", [128, 4 * H], bf16, kind="ExternalInput").ap()
        for s in scans
    ]
    d_bias = [
        nc.dram_tensor(f"bias_{s}", [128, NG], f32, kind="ExternalInput").ap()
        for s in scans
    ]
    d_x = [
        nc.dram_tensor(f"x_{s}", [t, FEAT, BL], bf16, kind="ExternalInput").ap()
        for s, t in zip(scans, (T_OBS, T_PRE))
    ]
    d_h0 = nc.dram_tensor("h0T", [H, BL], bf16, kind="ExternalInput").ap()
    d_c0 = nc.dram_tensor("c0T", [H, BL], f32, kind="ExternalInput").ap()
    d_c1 = nc.dram_tensor("c1T", [H, BL], f32, kind="ExternalInput").ap()
    d_hout = [
        nc.dram_tensor(f"h{i}T", [H, BL], f32, kind="ExternalOutput").ap()
        for i in (1, 2)
    ]

    with tile.TileContext(nc) as tc:
        with (
            tc.tile_pool(name="wp", bufs=1) as wp,
            tc.tile_pool(name="hp", bufs=18) as hp,
            tc.tile_pool(name="cp", bufs=1) as cp,
            tc.tile_pool(name="gf", bufs=3) as gf,
            tc.tile_pool(name="gb", bufs=12) as gb,
            tc.tile_pool(name="xp", bufs=3) as xp,
            tc.tile_pool(name="pp", bufs=8, space="PSUM") as pp,
        ):
            # Persistent weights. Scan-1 set loads in (j-chunk)-major order so
            # the first j-group can start after ~1.2MB; scan-2 set is emitted
            # inside the scan loop (after step 1) so its DMA traffic queues
            # behind the critical first-step loads.
            w_sb = [
                [
                    wp.tile([128, 4 * H], bf16, tag=f"w{s}_{k}", name=f"w{s}_{k}")
                    for k in range(KB)
                ]
                for s in range(2)
            ]
            wx_sb = [
                wp.tile([128, 4 * H], bf16, tag=f"wx{s}", name=f"wx{s}")
                for s in range(2)
            ]
            bias_sb = [
                wp.tile([128, NG], f32, tag=f"bias{s}", name=f"bias{s}")
                for s in range(2)
            ]

            def load_weight_set(s, fine_j0):
                # bias first (tiny), then j-group 0's working set as small
                # descriptors (so the first matmuls can start early), then
                # the rest as one big descriptor per k-tile: each descriptor
                # costs ~610ns of sequencer issue time, so fewer+bigger wins
                # once the critical first chunks are in flight.
                nc.sync.dma_start(out=bias_sb[s], in_=d_bias[s][:, :])
                j0 = slice(0, 512)
                rest = slice(512, 4 * H)
                if fine_j0:
                    nc.sync.dma_start(out=wx_sb[s][:, j0], in_=d_wx[s][:, j0])
                    for k in range(KB):
                        nc.sync.dma_start(
                            out=w_sb[s][k][:, j0],
                            in_=d_w[s][k * 128 : (k + 1) * 128, j0],
                        )
                    nc.sync.dma_start(out=wx_sb[s][:, rest], in_=d_wx[s][:, rest])
                    for k in range(KB):
                        nc.sync.dma_start(
                            out=w_sb[s][k][:, rest],
                            in_=d_w[s][k * 128 : (k + 1) * 128, rest],
                        )
                else:
                    nc.sync.dma_start(out=wx_sb[s], in_=d_wx[s][:, :])
                    for k in range(KB):
                        nc.sync.dma_start(
                            out=w_sb[s][k], in_=d_w[s][k * 128 : (k + 1) * 128, :]
                        )

            # PE warm-up: the array sits idle ~8us waiting for the first
            # weight/h0 DMAs, and would then start the real stream at the
            # HAM-throttled 1.2 GHz clock. Dummy matmuls on zeroed tiles
            # (never read) during that window flip the clock gate to 8/8
            # before the real stream begins (~3.4us of PE-busy needed).
            wu_w = xp.tile([128, 128], bf16, tag="x", name="wu_w")
            wu_rhs = xp.tile([128, BL], bf16, tag="x", name="wu_rhs")
            nc.vector.memset(wu_w, 0.0)
            nc.vector.memset(wu_rhs, 0.0)
            wu_p = pp.tile([128, BL], f32, tag="ps", name="wu_p")
            for _ in range(9):
                nc.tensor.matmul(wu_p, wu_w, wu_rhs, start=True, stop=True)

            load_weight_set(0, fine_j0=True)

            # h0 first (needed by the very first k-loop within ~2us of
            # compute start) as a single descriptor into one wide tile
            # (k-block j lands in columns j*BL..(j+1)*BL), then c0 (only
            # needed at activation time).
            h0_all = wp.tile([128, KB * BL], bf16, tag="h0a", name="h0_all")
            nc.scalar.dma_start(
                out=h0_all, in_=d_h0.rearrange("(k p) b -> p k b", k=KB)
            )
            h_cur = [h0_all[:, k * BL : (k + 1) * BL] for k in range(KB)]
            c_sb = [
                cp.tile([128, BL], f32, tag=f"c{j}", name=f"c{j}") for j in range(KB)
            ]
            for j in range(KB):
                nc.scalar.dma_start(out=c_sb[j], in_=d_c0[j * 128 : (j + 1) * 128, :])

            steps = [(0, t) for t in range(T_OBS)] + [(1, t) for t in range(T_PRE)]
            x_tiles = {}

            def load_x(s, t):
                if (s, t) in x_tiles:
                    return x_tiles[(s, t)]
                xt = xp.tile([128, BL], bf16, tag="x", name=f"x_{s}_{t}")
                for r in range(4):
                    nc.gpsimd.dma_start(
                        out=xt[32 * r : 32 * r + FEAT, :], in_=d_x[s][t, :, :]
                    )
                x_tiles[(s, t)] = xt
                return xt

            def emit_input_mms(s, t, g, xt):
                # four K=2 input-projection matmuls, row-packed into disjoint
                # 32-row strips -> concurrent on the PE. Starts group g's
                # PSUM accumulation.
                ps = []
                for gi in range(4):
                    m = 4 * g + gi
                    p = pp.tile([128, BL], f32, tag="ps", name=f"ps_{s}_{t}_{m}")
                    msl = slice(m * 128, (m + 1) * 128)
                    rsl = slice(32 * gi, 32 * gi + FEAT)
                    nc.tensor.matmul(
                        p,
                        wx_sb[s][rsl, msl],
                        xt[rsl, :],
                        start=True,
                        stop=False,
                        tile_position=(32 * gi, 0),
                        skip_group_check=True,
                    )
                    ps.append(p)
                return ps

            HA, HB = slice(0, HALF), slice(HALF, BL)
            pending = None  # psum tiles of the next group, emitted early

            for si_, (s, t) in enumerate(steps):
                if s == 0 and t == 1:
                    load_weight_set(1, fine_j0=False)
                xt = load_x(s, t)
                if si_ + 1 < len(steps):
                    load_x(*steps[si_ + 1])  # prefetch next step's x

                T = T_OBS if s == 0 else T_PRE
                last = t == T - 1
                produce_next = not (s == 1 and last)
                d_out = d_hout[s]
                h_next = [None] * KB

                for g in range(KB):
                    if pending is None:  # very first group only
                        pending = emit_input_mms(s, t, g, xt)
                    ps = pending
                    pending = None

                    ms = [4 * g + gi for gi in range(4)]
                    for gi in range(4):
                        msl = slice(ms[gi] * 128, (ms[gi] + 1) * 128)
                        for k in range(KB):
                            nc.tensor.matmul(
                                ps[gi],
                                w_sb[s][k][:, msl],
                                h_cur[k],
                                start=False,
                                stop=(k == KB - 1),
                                skip_group_check=True,
                            )
                            # hoist the next group's input MMs into the
                            # o-gate stream so their LDWEIGHTS don't burst
                            # at the group boundary
                            if gi == 3 and k == 0:
                                if g < KB - 1:
                                    pending = emit_input_mms(s, t, g + 1, xt)
                                elif si_ + 1 < len(steps):
                                    s2, t2 = steps[si_ + 1]
                                    pending = emit_input_mms(
                                        s2, t2, 0, x_tiles[(s2, t2)]
                                    )
                    pf, pi, pg, po = ps
                    bs = bias_sb[s]
                    # only the last group's chain gates the next step's final
                    # k-block matmul; halve its chunks to shorten that
                    # latency. Full-width elsewhere (each ScalarE instruction
                    # has ~220ns fixed overhead).
                    chunks = (HA, HB) if g == KB - 1 else (slice(0, BL),)

                    def _act(p_in, m, func, nm, hs):
                        o = _act.tiles.get(nm)
                        if o is None:
                            o = gb.tile([128, BL], bf16, tag="g", name=nm)
                            _act.tiles[nm] = o
                        nc.scalar.activation(
                            out=o[:, hs], in_=p_in[:, hs], func=func,
                            bias=bs[:, m : m + 1],
                        )
                        return o

                    _act.tiles = {}
                    # f,i,g gate activations (bf16 out)
                    for hs in chunks:
                        sf = _act(pf, ms[0], SIG, f"sf_{s}_{t}_{g}", hs)
                    for hs in chunks:
                        si = _act(pi, ms[1], SIG, f"si_{s}_{t}_{g}", hs)
                    for hs in chunks:
                        tg = _act(pg, ms[2], TANH, f"tg_{s}_{t}_{g}", hs)

                    # cell update: c = sf*c + si*tg (c stays fp32)
                    t1 = gf.tile([128, BL], f32, tag="t", name=f"t1_{s}_{t}_{g}")
                    t2 = gb.tile([128, BL], bf16, tag="g", name=f"t2_{s}_{t}_{g}")
                    for hs in chunks:
                        nc.vector.tensor_mul(t1[:, hs], sf[:, hs], c_sb[g][:, hs])
                        nc.vector.tensor_mul(t2[:, hs], si[:, hs], tg[:, hs])
                        nc.vector.tensor_add(c_sb[g][:, hs], t1[:, hs], t2[:, hs])

                    # o-gate + tanh(c); in the halved case so_a/tc_a complete
                    # before so_b/tc_b so the first half of h_next lands early
                    so = None
                    tc_j = gb.tile([128, BL], bf16, tag="g", name=f"tc_{s}_{t}_{g}")
                    h_n = (
                        hp.tile([128, BL], bf16, tag="h", name=f"h_{s}_{t}_{g}")
                        if produce_next
                        else None
                    )
                    hf = (
                        gf.tile([128, BL], f32, tag="t", name=f"hf_{s}_{g}")
                        if last
                        else None
                    )
                    for hs in chunks:
                        so = _act(po, ms[3], SIG, f"so_{s}_{t}_{g}", hs)
                        nc.scalar.activation(
                            out=tc_j[:, hs], in_=c_sb[g][:, hs], func=TANH
                        )
                        # next-step h first: it is on the critical path; the
                        # fp32 output copy and its DMA are not.
                        if produce_next:
                            nc.vector.tensor_mul(h_n[:, hs], so[:, hs], tc_j[:, hs])
                        if last:
                            nc.vector.tensor_mul(hf[:, hs], so[:, hs], tc_j[:, hs])
                    if last:
                        nc.sync.dma_start(
                            out=d_out[g * 128 : (g + 1) * 128, :], in_=hf
                        )
                    if produce_next:
                        h_next[g] = h_n

                    # refresh cell state for scan 2 as soon as scan 1 is done
                    # reading this j-block (lands during the remaining steps)
                    if s == 0 and last:
                        nc.scalar.dma_start(
                            out=c_sb[g], in_=d_c1[g * 128 : (g + 1) * 128, :]
                        )

                if produce_next:
                    h_cur = h_next

    nc.compile()
    return nc


def _prep_host(inputs):
    inputs = {k: np.asarray(v) for k, v in inputs.items()}
    f32 = np.float32
    W_in = inputs["W_in"].astype(np.float64)
    b_in = inputs["b_in"].astype(np.float64)

    shared = {}
    for tag in ("obs", "pre"):
        W_ih = inputs[f"W_ih_{tag}"].astype(np.float64)
        W_hh = inputs[f"W_hh_{tag}"].astype(f32)
        b = inputs[f"b_{tag}"].astype(np.float64)
        W_eff = (W_ih @ W_in).astype(f32)        # [4H, FEAT]
        b_eff = (W_ih @ b_in + b).astype(f32)    # [4H]
        # permute gate blocks into j-grouped (f,i,g,o) order
        w_p = W_hh.T.reshape(H, NG, 128)[:, _PERM, :].reshape(H, 4 * H)
        wx_p = np.zeros((128, 4 * H), f32)
        for r in range(4):
            wx_p[32 * r : 32 * r + FEAT] = (
                W_eff.T.reshape(FEAT, NG, 128)[:, _PERM, :].reshape(FEAT, 4 * H)
            )
        bias_p = b_eff.reshape(NG, 128)[_PERM, :].T  # [128, NG]
        shared[f"w_{tag}"] = np.ascontiguousarray(w_p).astype(_BF16)
        shared[f"wx_{tag}"] = np.ascontiguousarray(wx_p).astype(_BF16)
        shared[f"bias_{tag}"] = np.ascontiguousarray(bias_p)

    obs = inputs["obs_traj_rel"].astype(f32)
    pre = inputs["pre_traj_rel"].astype(f32)
    h0 = inputs["h0"].astype(f32)
    c0 = inputs["c0"].astype(f32)
    c1 = inputs["c1"].astype(f32)

    in_maps = []
    for c in range(N_CORES):
        sl = slice(c * BL, (c + 1) * BL)
        m = dict(shared)
        m["x_obs"] = np.ascontiguousarray(obs[:, sl, :].transpose(0, 2, 1)).astype(
            _BF16
        )
        m["x_pre"] = np.ascontiguousarray(pre[:, sl, :].transpose(0, 2, 1)).astype(
            _BF16
        )
        m["h0T"] = np.ascontiguousarray(h0[sl].T).astype(_BF16)
        m["c0T"] = np.ascontiguousarray(c0[sl].T)
        m["c1T"] = np.ascontiguousarray(c1[sl].T)
        in_maps.append(m)
    return in_maps


def _run(inputs, trace=False):
    from concourse import bass_utils

    nc = _CACHE.get("nc")
    if nc is None:
        nc = _build_nc()
        _CACHE["nc"] = nc
    in_maps = _prep_host(inputs)
    res = bass_utils.run_bass_kernel_spmd(
        nc, in_maps, core_ids=list(range(N_CORES)), trace=trace
    )
    h1 = np.concatenate([r["h1T"] for r in res.results], axis=1)  # [H, B] == h1.T
    h2 = np.concatenate([r["h2T"] for r in res.results], axis=1)
    c_out = np.ascontiguousarray(h1.reshape(B, H), dtype=np.float32)
    x_out = np.ascontiguousarray(h2.reshape(B, H), dtype=np.float32)
    return (c_out, x_out), res


def kernel(**inputs):
    out, _ = _run(inputs, trace=False)
    return out
